# revision 48
# baseline (speedup 1.0000x reference)
"""Trainium2 Bass kernel for GNN mean-aggregation message passing.

  m = relu(concat(y[src], ex) @ W1.T + b1)        per edge
  z = segment_mean(m, dst)                        per node (0 for isolated)
  h = relu(z @ W2.T + b2)                         per node

Strategy (8 NeuronCores, one SPMD program, edge-parallel by dst range):
  - Host shards edges by dst node range (N/8 nodes per core) and sorts each
    core's edges by (dst-window, dst). Per-window tile counts are unified
    across cores (max), so a single program fits all shards; padding slots
    carry all-zero features and zero one-hot scatter weight.
  - Host materializes per-edge features featT = [y[src]; ex; 1] (bf16) in
    edge-slot order, K-packed: a PAIR of 128-edge tiles is stacked into 98
    feature rows so one matmul against a block-diagonal [98, 96] weight
    computes both tiles' messages (halves PE instruction count).
  - The device runs the whole MLP + aggregation: per tile pair,
    m = relu(feat_pair.T @ W1b2) on PE+ACT, then per tile a scatter
    one-hot matmul s.T[48, win] += m.T @ O accumulates the segment sum in
    PSUM (O is 0/1 in fp8, exact; dst-sorted tiles make O spans ~10 cols).
    Scatters are emitted one super-batch behind the W1 matmuls so the ACT
    relu latency is hidden.
  - Window drain: z = s * (1/deg) (DVE, bf16 1/deg), h.T = relu(W2.T @
    z.T + b2) (PE+ACT), DMA out. 0-degree nodes yield 0 via zero sums.
"""

import os

import numpy as np
import ml_dtypes

# timing-ablation knob (empty in production): subset of
# {"w1", "act", "scatter", "feat", "drain", "odma", "cinvdma"}
DISABLE = set(os.environ.get("KDISABLE", "").split(",")) - {""}
# fp8 edge features (halves the dominant input tensor; ~1.3% rel err,
# within the 2e-2 budget). FEAT_FP8=0 falls back to bf16.
FEAT_FP8 = os.environ.get("FEAT_FP8", "1") == "1"

N_CORES = 8
WIN = 1024         # nodes per PSUM scatter window (2 banks)
TILE_E = 128       # edges per tile (PE contraction dim for scatter)
SUPER = 8          # tiles (= 4 K-packed pairs) per PSUM-m / ACT relu batch

BF16 = ml_dtypes.bfloat16
REPEAT = 1  # run the body N times (timing experiments only)


def _preprocess(y, ex, W1, b1, W2, b2, src, dst):
    N, ND = y.shape
    E, ED = ex.shape
    D = ND + ED
    K = D + 1  # feature rows incl. bias-ones row
    NPC = N // N_CORES
    NW = (NPC + WIN - 1) // WIN

    cnt = np.bincount(dst, minlength=N)
    inv_cnt = (1.0 / np.maximum(cnt, 1)).astype(np.float32)

    core_of = (dst // NPC).astype(np.int64)
    win_of = ((dst - core_of * NPC) // WIN).astype(np.int64)
    cw = core_of * NW + win_of
    key = cw * np.int64(N + 1) + dst
    order = np.argsort(key, kind="stable")

    dst_s = dst[order]
    src_s = src[order]
    ex_s = ex[order]
    core_s = core_of[order]
    win_s = win_of[order]
    cw_s = cw[order]

    cw_cnt = np.bincount(cw_s, minlength=N_CORES * NW).reshape(N_CORES, NW)
    # tiles per window, rounded to even (W1 matmuls process tile PAIRS)
    T_w = 2 * ((cw_cnt.max(axis=0) + 2 * TILE_E - 1) // (2 * TILE_E))  # [NW]
    win_block_base = np.concatenate([[0], np.cumsum(T_w)])
    B_tot = int(win_block_base[-1])
    E_slots = B_tot * TILE_E

    # rank of each edge within its (core, window) run
    cw_start = np.zeros(N_CORES * NW + 1, np.int64)
    cw_start[1:] = np.cumsum(cw_cnt.reshape(-1))
    rank = np.arange(E, dtype=np.int64) - cw_start[cw_s]
    slot = win_block_base[win_s] * TILE_E + rank
    tile_of = slot // TILE_E
    p_in_tile = slot % TILE_E

    # per-tile dst span (window-relative), unioned over cores
    rel = dst_s - core_s * NPC - win_s * WIN
    lo_t = np.full(B_tot, np.int64(1 << 60))
    hi_t = np.full(B_tot, np.int64(-1))
    np.minimum.at(lo_t, tile_of, rel)
    np.maximum.at(hi_t, tile_of, rel)
    empty = hi_t < 0
    lo_t[empty] = 0
    hi_t[empty] = 0
    span_t = hi_t - lo_t + 1
    col_off = np.concatenate([[0], np.cumsum(span_t)])
    C_tot = int(col_off[-1])
    o_col = col_off[tile_of] + (rel - lo_t[tile_of])

    # K-packed features: tile pair (2p, 2p+1) stacked into 98 rows so one
    # matmul with a block-diagonal [98, 96] weight computes both tiles' m.
    # Column c of pair p carries edge slots 2p*128+c (rows 0:49) and
    # (2p+1)*128+c (rows 49:98).
    half = slot // TILE_E % 2
    pcol = (slot // (2 * TILE_E)) * TILE_E + slot % TILE_E
    FDT = ml_dtypes.float8_e4m3 if FEAT_FP8 else BF16
    featT = np.zeros((N_CORES, 2 * K, E_slots // 2), FDT)
    O_a = np.zeros((N_CORES, TILE_E, C_tot), ml_dtypes.float8_e4m3)
    y_bf = y.astype(FDT)
    ex_bf = ex_s.astype(FDT)
    for c in range(N_CORES):
        m = core_s == c
        base = half[m] * K
        pc = pcol[m]
        for r in range(ND):
            featT[c, base + r, pc] = y_bf[src_s[m], r]
        for r in range(ED):
            featT[c, base + ND + r, pc] = ex_bf[m, r]
        featT[c, base + D, pc] = 1.0
        O_a[c, p_in_tile[m], o_col[m]] = 1.0

    cinv = np.empty((N_CORES, D, NPC), BF16)
    for c in range(N_CORES):
        cinv[c] = np.broadcast_to(inv_cnt[c * NPC : (c + 1) * NPC], (D, NPC))

    meta = {
        "N": N, "E": E, "ND": ND, "ED": ED, "D": D, "K": K, "NPC": NPC,
        "n_win": NW, "T_w": T_w, "win_block_base": win_block_base,
        "B_tot": B_tot, "E_slots": E_slots, "C_tot": C_tot,
        "lo_t": lo_t, "span_t": span_t, "col_off": col_off,
    }
    w1b = np.concatenate([W1.T, b1[None, :]], 0).astype(BF16)      # [49, 48]
    w1b2 = np.zeros((2 * K, 2 * D), BF16)                          # [98, 96]
    w1b2[:K, :D] = w1b
    w1b2[K:, D:] = w1b
    w2b = np.ascontiguousarray(W2.T).astype(np.float32)            # [48, 32]
    b2c = np.ascontiguousarray(b2.reshape(-1, 1)).astype(np.float32)

    # Pack every input into ONE u8 tensor per core: each jit-call operand
    # costs ~70 us of dispatch in this environment, so 6 inputs -> 1.
    # feat/O/cinv are stored as per-window contiguous blocks (512B-aligned)
    # so the per-window DMAs stay single contiguous slices.
    ALIGN = 512
    fsz = featT.itemsize
    cur = 0
    feat_off, o_off, cinv_off = [], [], []
    for w in range(NW):
        feat_off.append(cur)
        cur += -(2 * K * (int(T_w[w]) * TILE_E // 2) * fsz) // ALIGN * -ALIGN
    for w in range(NW):
        cn = int(col_off[win_block_base[w + 1]] - col_off[win_block_base[w]])
        o_off.append(cur)
        cur += -(TILE_E * cn) // ALIGN * -ALIGN
    for w in range(NW):
        wn = min(WIN, NPC - w * WIN)
        cinv_off.append(cur)
        cur += -(D * wn * 2) // ALIGN * -ALIGN
    w1b2_off = cur
    cur += -(w1b2.nbytes) // ALIGN * -ALIGN
    w2b_off = cur
    cur += -(w2b.nbytes) // ALIGN * -ALIGN
    b2_off = cur
    cur += -(b2c.nbytes) // ALIGN * -ALIGN
    NB = cur

    pk = np.zeros((N_CORES, NB), np.uint8)
    for c in range(N_CORES):
        for w in range(NW):
            b0 = int(win_block_base[w])
            T = int(T_w[w])
            e0, gn = b0 * TILE_E, T * TILE_E
            blk = pk[c, feat_off[w] : feat_off[w] + 2 * K * (gn // 2) * fsz]
            blk.view(FDT).reshape(2 * K, gn // 2)[:] = (
                featT[c, :, e0 // 2 : (e0 + gn) // 2])
            c0 = int(col_off[b0])
            cn = int(col_off[b0 + T]) - c0
            blk = pk[c, o_off[w] : o_off[w] + TILE_E * cn]
            blk.view(ml_dtypes.float8_e4m3).reshape(TILE_E, cn)[:] = (
                O_a[c, :, c0 : c0 + cn])
            wn = min(WIN, NPC - w * WIN)
            blk = pk[c, cinv_off[w] : cinv_off[w] + D * wn * 2]
            blk.view(BF16).reshape(D, wn)[:] = (
                cinv[c, :, w * WIN : w * WIN + wn])
        pk[c, w1b2_off : w1b2_off + w1b2.nbytes].view(BF16).reshape(
            w1b2.shape)[:] = w1b2
        pk[c, w2b_off : w2b_off + w2b.nbytes].view(np.float32).reshape(
            w2b.shape)[:] = w2b
        pk[c, b2_off : b2_off + b2c.nbytes].view(np.float32).reshape(
            b2c.shape)[:] = b2c

    meta.update(feat_off=feat_off, o_off=o_off, cinv_off=cinv_off,
                w1b2_off=w1b2_off, w2b_off=w2b_off, b2_off=b2_off, NB=NB)
    consts = dict()
    per_core = dict(pk=pk)
    return consts, per_core, meta


def _split_excess_waits(nc, mybir):
    """This walrus build accepts at most 1 sync wait per instruction (0 on
    Drain). Move extras onto NOPs inserted just before, same engine."""
    for fn in nc.m.functions:
        for bb in fn.blocks:
            new_list = []
            for ins in bb.instructions:
                si = ins.sync_info
                limit = 0 if isinstance(ins, mybir.InstDrain) else 1
                if si is not None and si.on_wait and len(si.on_wait) > limit:
                    waits = list(si.on_wait)
                    keep, extra = waits[:limit], waits[limit:]
                    while extra:
                        chunk, extra = extra[:1], extra[1:]
                        nop = mybir.InstNoOp(
                            name=nc.get_next_instruction_name(), ins=[], outs=[])
                        nop.engine = ins.engine
                        nop.sync_info = mybir.SyncInfo(on_wait=chunk, on_update=[])
                        nc.register_instruction(nop)
                        new_list.append(nop)
                    si.on_wait = keep
                new_list.append(ins)
            bb.instructions[:] = new_list


def _build_program(meta):
    import concourse.bacc as bacc
    import concourse.mybir as mybir
    import concourse.tile as tile

    f32 = mybir.dt.float32
    bf16 = mybir.dt.bfloat16
    f8 = mybir.dt.float8e4
    Relu = mybir.ActivationFunctionType.Relu
    MULT = mybir.AluOpType.mult

    D, K, NPC, NW = meta["D"], meta["K"], meta["NPC"], meta["n_win"]
    T_w, wbb = meta["T_w"], meta["win_block_base"]
    E_slots, C_tot = meta["E_slots"], meta["C_tot"]
    lo_t, span_t, col_off = meta["lo_t"], meta["span_t"], meta["col_off"]
    OD = 32

    fdt = f8 if FEAT_FP8 else bf16
    nc = bacc.Bacc("TRN2")
    pk_ext = nc.dram_tensor("pk", [1, meta["NB"]], mybir.dt.uint8,
                            kind="ExternalInput")
    out_ext = nc.dram_tensor("hT", [OD, NPC], f32, kind="ExternalOutput")

    def pk_ap(off, nbytes, dtype, p):
        """[p, n] dtype view of packed bytes [off, off+nbytes)."""
        ap = pk_ext[0:1, off : off + nbytes].bitcast(dtype)
        return ap.rearrange("o (p n) -> (o p) n", p=p)

    with tile.TileContext(nc) as tc:
        with (
            tc.tile_pool(name="const", bufs=1) as cpool,
            tc.tile_pool(name="io", bufs=2) as iopool,
            tc.tile_pool(name="msb", bufs=4) as mpool,
            tc.tile_pool(name="psM", bufs=2, space="PSUM") as psM,
            tc.tile_pool(name="psZ", bufs=2, space="PSUM") as psZ,
            tc.tile_pool(name="psH", bufs=1, space="PSUM") as psH,
        ):
            w1b_sb = cpool.tile([2 * K, 2 * D], bf16)
            nc.sync.dma_start(out=w1b_sb[:], in_=pk_ap(
                meta["w1b2_off"], 2 * K * 2 * D * 2, bf16, 2 * K))
            w2b_sb = cpool.tile([D, OD], f32)
            nc.sync.dma_start(out=w2b_sb[:], in_=pk_ap(
                meta["w2b_off"], D * OD * 4, f32, D))
            b2_sb = cpool.tile([OD, 1], f32)
            nc.sync.dma_start(out=b2_sb[:], in_=pk_ap(
                meta["b2_off"], OD * 4, f32, OD))
            zl_bf = cpool.tile([1, D], bf16)
            nc.any.memset(zl_bf[:], 0)
            zr_bf = cpool.tile([1, 512], bf16)
            nc.any.memset(zr_bf[:], 0)

            for _rep in range(REPEAT):
                for w in range(NW):
                    T = int(T_w[w])
                    if T == 0:
                        continue
                    wn = min(WIN, NPC - w * WIN)
                    b0 = int(wbb[w])
                    e0 = b0 * TILE_E
                    gn = T * TILE_E
                    c0 = int(col_off[b0])
                    cn = int(col_off[b0 + T]) - c0

                    fb = 1 if FEAT_FP8 else 2
                    feat_t = iopool.tile([2 * K, gn // 2], fdt, tag="feat")
                    if "feat" not in DISABLE:
                        nc.sync.dma_start(
                            out=feat_t[:],
                            in_=pk_ap(meta["feat_off"][w],
                                      2 * K * (gn // 2) * fb, fdt, 2 * K))
                    o_t = iopool.tile([TILE_E, cn], f8, tag="omat")
                    if "odma" not in DISABLE:
                        nc.sync.dma_start(
                            out=o_t[:],
                            in_=pk_ap(meta["o_off"][w], TILE_E * cn, f8,
                                      TILE_E))
                    cinv_t = iopool.tile([D, WIN], bf16, tag="cinv")
                    if "cinvdma" not in DISABLE or w == NW - 1:
                        nc.sync.dma_start(
                            out=cinv_t[:, :wn],
                            in_=pk_ap(meta["cinv_off"][w], D * wn * 2, bf16,
                                      D))

                    psz = psZ.tile([D, WIN], f32, tag="psz")
                    for j in range(0, WIN, 512):
                        nc.tensor.matmul(
                            psz[:, j : j + 512], zl_bf[:], zr_bf[:],
                            start=True, stop=True)

                    def emit_scatter(m_sb, s, sb, final):
                        m_flat = m_sb.rearrange("p a b -> p (a b)")
                        for t in range(sb):
                            bt = b0 + s + t
                            mc = (t // 2) * 2 * D + (t % 2) * D
                            lo = int(lo_t[bt])
                            sp = int(span_t[bt])
                            off = int(col_off[bt]) - c0
                            last = final and t == sb - 1
                            # split at 512-col PSUM bank boundary
                            cuts = [lo, sp]
                            if lo // 512 != (lo + sp - 1) // 512:
                                sp1 = (lo // 512 + 1) * 512 - lo
                                cuts = [lo, sp1, lo + sp1, sp - sp1]
                            for k in range(0, len(cuts), 2):
                                clo, csp = cuts[k], cuts[k + 1]
                                if csp <= 0:
                                    continue
                                nc.tensor.matmul(
                                    psz[:, clo : clo + csp],
                                    m_flat[:, mc : mc + D],
                                    o_t[:, off + (clo - lo)
                                        : off + (clo - lo) + csp],
                                    start=False,
                                    stop=last and k + 2 >= len(cuts),
                                    skip_group_check=True)

                    # software-pipelined: scatter of super s is emitted after
                    # the W1 matmuls of super s+1, hiding the ACT relu latency.
                    # One W1 matmul computes a PAIR of tiles (K-packed feat,
                    # block-diag weight): out pair q at psum cols [128q,
                    # 128q+96) -- 128-col stride keeps pairs bank-aligned.
                    n_super = (T + SUPER - 1) // SUPER  # SUPER tiles = SUPER/2 pairs
                    NP = SUPER // 2
                    pending = None
                    for si in range(n_super):
                        s = si * SUPER
                        sb = min(SUPER, T - s)
                        np_q = sb // 2
                        ps_m = psM.tile([TILE_E, NP, TILE_E], f32, tag="psm")
                        for q in range(np_q if "w1" not in DISABLE else 0):
                            nc.tensor.matmul(
                                ps_m[:, q, : 2 * D],
                                feat_t[:, (s // 2 + q) * TILE_E
                                       : (s // 2 + q + 1) * TILE_E],
                                w1b_sb[:], start=True, stop=True)
                        m_sb = mpool.tile([TILE_E, NP, 2 * D], bf16, tag="m")
                        if "act" not in DISABLE and "w1" not in DISABLE:
                            nc.scalar.activation(
                                out=m_sb[:, :np_q, :],
                                in_=ps_m[:, :np_q, : 2 * D], func=Relu)
                        if pending is not None and "scatter" not in DISABLE:
                            emit_scatter(*pending, final=False)
                        pending = (m_sb, s, sb)
                    if "scatter" not in DISABLE:
                        emit_scatter(*pending, final=True)

                    if "drain" in DISABLE and w < NW - 1:
                        continue
                    zt = mpool.tile([D, WIN], f32, tag="zt")
                    nc.vector.tensor_tensor(
                        out=zt[:, :wn], in0=psz[:, :wn], in1=cinv_t[:, :wn],
                        op=MULT)
                    ps_h = psH.tile([OD, WIN], f32, tag="psh")
                    for j in range(0, wn, 512):
                        jn = min(512, wn - j)
                        nc.tensor.matmul(
                            ps_h[:, j : j + jn], w2b_sb[:], zt[:, j : j + jn],
                            start=True, stop=True)
                    h_sb = mpool.tile([OD, WIN], f32, tag="h")
                    nc.scalar.activation(
                        out=h_sb[:, :wn], in_=ps_h[:, :wn], func=Relu,
                        bias=b2_sb[:, 0:1])
                    nc.sync.dma_start(
                        out=out_ext[:, w * WIN : w * WIN + wn],
                        in_=h_sb[:, :wn])

    nc.compile()
    _split_excess_waits(nc, mybir)
    return nc


def build_in_maps(consts, per_core):
    return [{"pk": per_core["pk"][c][None, :]} for c in range(N_CORES)]


def kernel(y, ex, W1, b1, W2, b2, src, dst):
    from concourse.bass_utils import run_bass_kernel_spmd

    y = np.asarray(y, dtype=np.float32)
    ex = np.asarray(ex, dtype=np.float32)
    W1 = np.asarray(W1, dtype=np.float32)
    b1 = np.asarray(b1, dtype=np.float32)
    W2 = np.asarray(W2, dtype=np.float32)
    b2 = np.asarray(b2, dtype=np.float32)
    src = np.asarray(src, dtype=np.int32)
    dst = np.asarray(dst, dtype=np.int32)

    consts, per_core, meta = _preprocess(y, ex, W1, b1, W2, b2, src, dst)
    nc = _build_program(meta)
    in_maps = build_in_maps(consts, per_core)
    res = run_bass_kernel_spmd(nc, in_maps, list(range(N_CORES)))

    NPC = meta["NPC"]
    h = np.empty((meta["N"], 32), dtype=np.float32)
    for c in range(N_CORES):
        h[c * NPC : (c + 1) * NPC, :] = res.results[c]["hT"].T
    return h


# revision 49
# speedup vs baseline: 1.2227x; 1.2227x over previous
"""Trainium2 Bass kernel for GNN mean-aggregation message passing.

  m = relu(concat(y[src], ex) @ W1.T + b1)        per edge
  z = segment_mean(m, dst)                        per node (0 for isolated)
  h = relu(z @ W2.T + b2)                         per node

Strategy (8 NeuronCores, one SPMD program, edge-parallel by dst range):
  - Host shards edges by dst node range (N/8 nodes per core) and sorts each
    core's edges by (dst-window, dst). Per-window tile counts are unified
    across cores (max), so a single program fits all shards; padding slots
    carry all-zero features and zero one-hot scatter weight.
  - Host materializes per-edge features featT = [y[src]; ex; 1] (bf16) in
    edge-slot order, K-packed: a PAIR of 128-edge tiles is stacked into 98
    feature rows so one matmul against a block-diagonal [98, 96] weight
    computes both tiles' messages (halves PE instruction count).
  - The device runs the whole MLP + aggregation: per tile pair,
    m = relu(feat_pair.T @ W1b2) on PE+ACT, then per tile a scatter
    one-hot matmul s.T[48, win] += m.T @ O accumulates the segment sum in
    PSUM (O is 0/1 in fp8, exact; dst-sorted tiles make O spans ~10 cols).
    Scatters are emitted one super-batch behind the W1 matmuls so the ACT
    relu latency is hidden.
  - Window drain: z = s * (1/deg) (DVE, bf16 1/deg), h.T = relu(W2.T @
    z.T + b2) (PE+ACT), DMA out. 0-degree nodes yield 0 via zero sums.
"""

import os

import numpy as np
import ml_dtypes

# timing-ablation knob (empty in production): subset of
# {"w1", "act", "scatter", "feat", "drain", "odma", "cinvdma"}
DISABLE = set(os.environ.get("KDISABLE", "").split(",")) - {""}
# fp8 edge features (halves the dominant input tensor; ~1.3% rel err,
# within the 2e-2 budget). FEAT_FP8=0 falls back to bf16.
FEAT_FP8 = os.environ.get("FEAT_FP8", "1") == "1"

N_CORES = 8
WIN = 1024         # nodes per PSUM scatter window (2 banks)
TILE_E = 128       # edges per tile (PE contraction dim for scatter)
SUPER = 16         # tiles (= 8 K-packed pairs) per PSUM-m / ACT relu batch

BF16 = ml_dtypes.bfloat16
REPEAT = 1  # run the body N times (timing experiments only)


def _preprocess(y, ex, W1, b1, W2, b2, src, dst):
    N, ND = y.shape
    E, ED = ex.shape
    D = ND + ED
    K = D + 1  # feature rows incl. bias-ones row
    NPC = N // N_CORES
    NW = (NPC + WIN - 1) // WIN

    cnt = np.bincount(dst, minlength=N)
    inv_cnt = (1.0 / np.maximum(cnt, 1)).astype(np.float32)

    core_of = (dst // NPC).astype(np.int64)
    win_of = ((dst - core_of * NPC) // WIN).astype(np.int64)
    cw = core_of * NW + win_of
    key = cw * np.int64(N + 1) + dst
    order = np.argsort(key, kind="stable")

    dst_s = dst[order]
    src_s = src[order]
    ex_s = ex[order]
    core_s = core_of[order]
    win_s = win_of[order]
    cw_s = cw[order]

    cw_cnt = np.bincount(cw_s, minlength=N_CORES * NW).reshape(N_CORES, NW)
    # tiles per window, rounded to even (W1 matmuls process tile PAIRS)
    T_w = 2 * ((cw_cnt.max(axis=0) + 2 * TILE_E - 1) // (2 * TILE_E))  # [NW]
    win_block_base = np.concatenate([[0], np.cumsum(T_w)])
    B_tot = int(win_block_base[-1])
    E_slots = B_tot * TILE_E

    # rank of each edge within its (core, window) run
    cw_start = np.zeros(N_CORES * NW + 1, np.int64)
    cw_start[1:] = np.cumsum(cw_cnt.reshape(-1))
    rank = np.arange(E, dtype=np.int64) - cw_start[cw_s]
    slot = win_block_base[win_s] * TILE_E + rank
    tile_of = slot // TILE_E
    p_in_tile = slot % TILE_E

    # per-tile dst span (window-relative), unioned over cores
    rel = dst_s - core_s * NPC - win_s * WIN
    lo_t = np.full(B_tot, np.int64(1 << 60))
    hi_t = np.full(B_tot, np.int64(-1))
    np.minimum.at(lo_t, tile_of, rel)
    np.maximum.at(hi_t, tile_of, rel)
    empty = hi_t < 0
    lo_t[empty] = 0
    hi_t[empty] = 0
    span_t = hi_t - lo_t + 1
    col_off = np.concatenate([[0], np.cumsum(span_t)])
    C_tot = int(col_off[-1])
    o_col = col_off[tile_of] + (rel - lo_t[tile_of])

    # K-packed features: tile pair (2p, 2p+1) stacked into 98 rows so one
    # matmul with a block-diagonal [98, 96] weight computes both tiles' m.
    # Column c of pair p carries edge slots 2p*128+c (rows 0:49) and
    # (2p+1)*128+c (rows 49:98).
    half = slot // TILE_E % 2
    pcol = (slot // (2 * TILE_E)) * TILE_E + slot % TILE_E
    FDT = ml_dtypes.float8_e4m3 if FEAT_FP8 else BF16
    featT = np.zeros((N_CORES, 2 * K, E_slots // 2), FDT)
    O_a = np.zeros((N_CORES, TILE_E, C_tot), ml_dtypes.float8_e4m3)
    y_bf = y.astype(FDT)
    ex_bf = ex_s.astype(FDT)
    for c in range(N_CORES):
        m = core_s == c
        base = half[m] * K
        pc = pcol[m]
        for r in range(ND):
            featT[c, base + r, pc] = y_bf[src_s[m], r]
        for r in range(ED):
            featT[c, base + ND + r, pc] = ex_bf[m, r]
        featT[c, base + D, pc] = 1.0
        O_a[c, p_in_tile[m], o_col[m]] = 1.0

    cinv = np.empty((N_CORES, D, NPC), BF16)
    for c in range(N_CORES):
        cinv[c] = np.broadcast_to(inv_cnt[c * NPC : (c + 1) * NPC], (D, NPC))

    meta = {
        "N": N, "E": E, "ND": ND, "ED": ED, "D": D, "K": K, "NPC": NPC,
        "n_win": NW, "T_w": T_w, "win_block_base": win_block_base,
        "B_tot": B_tot, "E_slots": E_slots, "C_tot": C_tot,
        "lo_t": lo_t, "span_t": span_t, "col_off": col_off,
    }
    w1b = np.concatenate([W1.T, b1[None, :]], 0).astype(BF16)      # [49, 48]
    w1b2 = np.zeros((2 * K, 2 * D), BF16)                          # [98, 96]
    w1b2[:K, :D] = w1b
    w1b2[K:, D:] = w1b
    w2b = np.ascontiguousarray(W2.T).astype(np.float32)            # [48, 32]
    b2c = np.ascontiguousarray(b2.reshape(-1, 1)).astype(np.float32)

    # Pack every input into ONE u8 tensor per core: each jit-call operand
    # costs ~70 us of dispatch in this environment, so 6 inputs -> 1.
    # feat/O/cinv are stored as per-window contiguous blocks (512B-aligned)
    # so the per-window DMAs stay single contiguous slices.
    ALIGN = 512
    fsz = featT.itemsize
    cur = 0
    feat_off, o_off, cinv_off = [], [], []
    for w in range(NW):
        feat_off.append(cur)
        cur += -(2 * K * (int(T_w[w]) * TILE_E // 2) * fsz) // ALIGN * -ALIGN
    for w in range(NW):
        cn = int(col_off[win_block_base[w + 1]] - col_off[win_block_base[w]])
        o_off.append(cur)
        cur += -(TILE_E * cn) // ALIGN * -ALIGN
    for w in range(NW):
        wn = min(WIN, NPC - w * WIN)
        cinv_off.append(cur)
        cur += -(D * wn * 2) // ALIGN * -ALIGN
    w1b2_off = cur
    cur += -(w1b2.nbytes) // ALIGN * -ALIGN
    w2b_off = cur
    cur += -(w2b.nbytes) // ALIGN * -ALIGN
    b2_off = cur
    cur += -(b2c.nbytes) // ALIGN * -ALIGN
    NB = cur

    pk = np.zeros((N_CORES, NB), np.uint8)
    for c in range(N_CORES):
        for w in range(NW):
            b0 = int(win_block_base[w])
            T = int(T_w[w])
            e0, gn = b0 * TILE_E, T * TILE_E
            blk = pk[c, feat_off[w] : feat_off[w] + 2 * K * (gn // 2) * fsz]
            blk.view(FDT).reshape(2 * K, gn // 2)[:] = (
                featT[c, :, e0 // 2 : (e0 + gn) // 2])
            c0 = int(col_off[b0])
            cn = int(col_off[b0 + T]) - c0
            blk = pk[c, o_off[w] : o_off[w] + TILE_E * cn]
            blk.view(ml_dtypes.float8_e4m3).reshape(TILE_E, cn)[:] = (
                O_a[c, :, c0 : c0 + cn])
            wn = min(WIN, NPC - w * WIN)
            blk = pk[c, cinv_off[w] : cinv_off[w] + D * wn * 2]
            blk.view(BF16).reshape(D, wn)[:] = (
                cinv[c, :, w * WIN : w * WIN + wn])
        pk[c, w1b2_off : w1b2_off + w1b2.nbytes].view(BF16).reshape(
            w1b2.shape)[:] = w1b2
        pk[c, w2b_off : w2b_off + w2b.nbytes].view(np.float32).reshape(
            w2b.shape)[:] = w2b
        pk[c, b2_off : b2_off + b2c.nbytes].view(np.float32).reshape(
            b2c.shape)[:] = b2c

    meta.update(feat_off=feat_off, o_off=o_off, cinv_off=cinv_off,
                w1b2_off=w1b2_off, w2b_off=w2b_off, b2_off=b2_off, NB=NB)
    consts = dict()
    per_core = dict(pk=pk)
    return consts, per_core, meta


def _split_excess_waits(nc, mybir):
    """This walrus build accepts at most 1 sync wait per instruction (0 on
    Drain). Move extras onto NOPs inserted just before, same engine."""
    for fn in nc.m.functions:
        for bb in fn.blocks:
            new_list = []
            for ins in bb.instructions:
                si = ins.sync_info
                limit = 0 if isinstance(ins, mybir.InstDrain) else 1
                if si is not None and si.on_wait and len(si.on_wait) > limit:
                    waits = list(si.on_wait)
                    keep, extra = waits[:limit], waits[limit:]
                    while extra:
                        chunk, extra = extra[:1], extra[1:]
                        nop = mybir.InstNoOp(
                            name=nc.get_next_instruction_name(), ins=[], outs=[])
                        nop.engine = ins.engine
                        nop.sync_info = mybir.SyncInfo(on_wait=chunk, on_update=[])
                        nc.register_instruction(nop)
                        new_list.append(nop)
                    si.on_wait = keep
                new_list.append(ins)
            bb.instructions[:] = new_list


def _build_program(meta):
    import concourse.bacc as bacc
    import concourse.mybir as mybir
    import concourse.tile as tile

    f32 = mybir.dt.float32
    bf16 = mybir.dt.bfloat16
    f8 = mybir.dt.float8e4
    Relu = mybir.ActivationFunctionType.Relu
    MULT = mybir.AluOpType.mult

    D, K, NPC, NW = meta["D"], meta["K"], meta["NPC"], meta["n_win"]
    T_w, wbb = meta["T_w"], meta["win_block_base"]
    E_slots, C_tot = meta["E_slots"], meta["C_tot"]
    lo_t, span_t, col_off = meta["lo_t"], meta["span_t"], meta["col_off"]
    OD = 32

    fdt = f8 if FEAT_FP8 else bf16
    nc = bacc.Bacc("TRN2")
    pk_ext = nc.dram_tensor("pk", [1, meta["NB"]], mybir.dt.uint8,
                            kind="ExternalInput")
    out_ext = nc.dram_tensor("hT", [OD, NPC], f32, kind="ExternalOutput")

    def pk_ap(off, nbytes, dtype, p):
        """[p, n] dtype view of packed bytes [off, off+nbytes)."""
        ap = pk_ext[0:1, off : off + nbytes].bitcast(dtype)
        return ap.rearrange("o (p n) -> (o p) n", p=p)

    with tile.TileContext(nc) as tc:
        with (
            tc.tile_pool(name="const", bufs=1) as cpool,
            tc.tile_pool(name="io", bufs=2) as iopool,
            tc.tile_pool(name="msb", bufs=4) as mpool,
            tc.tile_pool(name="psM", bufs=2, space="PSUM") as psM,
            tc.tile_pool(name="psZ", bufs=1, space="PSUM") as psZ,
            tc.tile_pool(name="psH", bufs=1, space="PSUM") as psH,
        ):
            w1b_sb = cpool.tile([2 * K, 2 * D], bf16)
            nc.sync.dma_start(out=w1b_sb[:], in_=pk_ap(
                meta["w1b2_off"], 2 * K * 2 * D * 2, bf16, 2 * K))
            w2b_sb = cpool.tile([D, OD], f32)
            nc.sync.dma_start(out=w2b_sb[:], in_=pk_ap(
                meta["w2b_off"], D * OD * 4, f32, D))
            b2_sb = cpool.tile([OD, 1], f32)
            nc.sync.dma_start(out=b2_sb[:], in_=pk_ap(
                meta["b2_off"], OD * 4, f32, OD))
            zl_bf = cpool.tile([1, D], bf16)
            nc.any.memset(zl_bf[:], 0)
            zr_bf = cpool.tile([1, 512], bf16)
            nc.any.memset(zr_bf[:], 0)

            for _rep in range(REPEAT):
                for w in range(NW):
                    T = int(T_w[w])
                    if T == 0:
                        continue
                    wn = min(WIN, NPC - w * WIN)
                    b0 = int(wbb[w])
                    e0 = b0 * TILE_E
                    gn = T * TILE_E
                    c0 = int(col_off[b0])
                    cn = int(col_off[b0 + T]) - c0

                    fb = 1 if FEAT_FP8 else 2
                    feat_t = iopool.tile([2 * K, gn // 2], fdt, tag="feat")
                    if "feat" not in DISABLE:
                        nc.sync.dma_start(
                            out=feat_t[:],
                            in_=pk_ap(meta["feat_off"][w],
                                      2 * K * (gn // 2) * fb, fdt, 2 * K))
                    o_t = iopool.tile([TILE_E, cn], f8, tag="omat")
                    if "odma" not in DISABLE:
                        nc.sync.dma_start(
                            out=o_t[:],
                            in_=pk_ap(meta["o_off"][w], TILE_E * cn, f8,
                                      TILE_E))
                    cinv_t = iopool.tile([D, WIN], bf16, tag="cinv")
                    if "cinvdma" not in DISABLE or w == NW - 1:
                        nc.sync.dma_start(
                            out=cinv_t[:, :wn],
                            in_=pk_ap(meta["cinv_off"][w], D * wn * 2, bf16,
                                      D))

                    psz = psZ.tile([D, WIN], f32, tag="psz")
                    for j in range(0, WIN, 512):
                        nc.tensor.matmul(
                            psz[:, j : j + 512], zl_bf[:], zr_bf[:],
                            start=True, stop=True)

                    def emit_scatter(m_sb, s, sb, final):
                        m_flat = m_sb.rearrange("p a b -> p (a b)")
                        for t in range(sb):
                            bt = b0 + s + t
                            mc = (t // 2) * 2 * D + (t % 2) * D
                            lo = int(lo_t[bt])
                            sp = int(span_t[bt])
                            off = int(col_off[bt]) - c0
                            last = final and t == sb - 1
                            # split at 512-col PSUM bank boundary
                            cuts = [lo, sp]
                            if lo // 512 != (lo + sp - 1) // 512:
                                sp1 = (lo // 512 + 1) * 512 - lo
                                cuts = [lo, sp1, lo + sp1, sp - sp1]
                            for k in range(0, len(cuts), 2):
                                clo, csp = cuts[k], cuts[k + 1]
                                if csp <= 0:
                                    continue
                                nc.tensor.matmul(
                                    psz[:, clo : clo + csp],
                                    m_flat[:, mc : mc + D],
                                    o_t[:, off + (clo - lo)
                                        : off + (clo - lo) + csp],
                                    start=False,
                                    stop=last and k + 2 >= len(cuts),
                                    skip_group_check=True)

                    # software-pipelined: scatter of super s is emitted after
                    # the W1 matmuls of super s+1, hiding the ACT relu latency.
                    # One W1 matmul computes a PAIR of tiles (K-packed feat,
                    # block-diag weight): out pair q at psum cols [128q,
                    # 128q+96) -- 128-col stride keeps pairs bank-aligned.
                    n_super = (T + SUPER - 1) // SUPER  # SUPER tiles = SUPER/2 pairs
                    NP = SUPER // 2
                    pending = None
                    for si in range(n_super):
                        s = si * SUPER
                        sb = min(SUPER, T - s)
                        np_q = sb // 2
                        ps_m = psM.tile([TILE_E, NP, TILE_E], f32, tag="psm")
                        for q in range(np_q if "w1" not in DISABLE else 0):
                            nc.tensor.matmul(
                                ps_m[:, q, : 2 * D],
                                feat_t[:, (s // 2 + q) * TILE_E
                                       : (s // 2 + q + 1) * TILE_E],
                                w1b_sb[:], start=True, stop=True)
                        m_sb = mpool.tile([TILE_E, NP, 2 * D], bf16, tag="m")
                        if "act" not in DISABLE and "w1" not in DISABLE:
                            nc.scalar.activation(
                                out=m_sb[:, :np_q, :],
                                in_=ps_m[:, :np_q, : 2 * D], func=Relu)
                        if pending is not None and "scatter" not in DISABLE:
                            emit_scatter(*pending, final=False)
                        pending = (m_sb, s, sb)
                    if "scatter" not in DISABLE:
                        emit_scatter(*pending, final=True)

                    if "drain" in DISABLE and w < NW - 1:
                        continue
                    zt = mpool.tile([D, WIN], f32, tag="zt")
                    nc.vector.tensor_tensor(
                        out=zt[:, :wn], in0=psz[:, :wn], in1=cinv_t[:, :wn],
                        op=MULT)
                    ps_h = psH.tile([OD, WIN], f32, tag="psh")
                    for j in range(0, wn, 512):
                        jn = min(512, wn - j)
                        nc.tensor.matmul(
                            ps_h[:, j : j + jn], w2b_sb[:], zt[:, j : j + jn],
                            start=True, stop=True)
                    h_sb = mpool.tile([OD, WIN], f32, tag="h")
                    nc.scalar.activation(
                        out=h_sb[:, :wn], in_=ps_h[:, :wn], func=Relu,
                        bias=b2_sb[:, 0:1])
                    nc.sync.dma_start(
                        out=out_ext[:, w * WIN : w * WIN + wn],
                        in_=h_sb[:, :wn])

    nc.compile()
    _split_excess_waits(nc, mybir)
    return nc


def build_in_maps(consts, per_core):
    return [{"pk": per_core["pk"][c][None, :]} for c in range(N_CORES)]


def kernel(y, ex, W1, b1, W2, b2, src, dst):
    from concourse.bass_utils import run_bass_kernel_spmd

    y = np.asarray(y, dtype=np.float32)
    ex = np.asarray(ex, dtype=np.float32)
    W1 = np.asarray(W1, dtype=np.float32)
    b1 = np.asarray(b1, dtype=np.float32)
    W2 = np.asarray(W2, dtype=np.float32)
    b2 = np.asarray(b2, dtype=np.float32)
    src = np.asarray(src, dtype=np.int32)
    dst = np.asarray(dst, dtype=np.int32)

    consts, per_core, meta = _preprocess(y, ex, W1, b1, W2, b2, src, dst)
    nc = _build_program(meta)
    in_maps = build_in_maps(consts, per_core)
    res = run_bass_kernel_spmd(nc, in_maps, list(range(N_CORES)))

    NPC = meta["NPC"]
    h = np.empty((meta["N"], 32), dtype=np.float32)
    for c in range(N_CORES):
        h[c * NPC : (c + 1) * NPC, :] = res.results[c]["hT"].T
    return h
